# revision 9
# baseline (speedup 1.0000x reference)
"""Trainium2 Bass kernel for a 13-layer causal dilated conv stack with gating.

Model (per reference):
    Wx_f = 13 causal dilated convs (K=2, dilation 2^i) over x with Wf
    Wx_g = same with Wg
    out  = tanh(Wx_f + h@Vf) * sigmoid(Wx_g + h@Vg)

Shapes: x (16, 8192, 64) f32, h (16, 64), Wf/Wg (13, 2, 64, 64), Vf/Vg (64, 64).

Strategy:
  - Data-parallel over batch: 2 batch elements per core on 8 cores, no
    collectives.
  - On-chip layout [128 partitions = (b*64 + c), T free]: both local batch
    elements' channels stacked on the partition axis. The host pre-transposes
    x into this channel-major layout (and transposes the output back), so the
    device does no layout changes at all.
  - Each conv layer is, per 512-token tile, two accumulating PE matmuls (one
    per tap) with block-diagonal kron(I2, W[tap]) stationary weights -> full
    128-wide PE utilization.
  - Causality: activation buffers carry a 256-column zero margin covering
    dilations < 512; for d >= 512 the boundary is tile-aligned and the tap-0
    matmul is simply skipped on the first d/512 tiles.
  - Full fp16 datapath (x, weights, inter-layer activations): full-rate PE
    like fp32r, but halves LDWEIGHTS SBUF traffic (the per-matmul weight
    reload is fully hidden at ~216ns matmul spacing vs ~227ns for f32r),
    halves input DMA, and keeps 10-bit mantissas so the 26-layer stack
    stays at ~4e-3 relative error (bf16's 8-bit mantissa fails the gate).
    PSUM accumulation is f32 throughout; mixing 16/32-bit matmul operands
    is not supported by the backend, so weights and activations must both
    be fp16.
  - h@V bias is computed on-device with kron(I2, V) and fused into the
    tanh/sigmoid activations via the ScalarE bias port.
  - PSUM->SBUF drain copies alternate DVE/DVE/ACT so neither engine
    bottlenecks the PE (scratch stays DVE-only for the output DMAs).
  - Startup: DMA issue order is consts -> f0 weights -> a small (512 col)
    first x chunk -> rest of the weight head -> remaining x chunks -> weight
    tail, so the first conv matmul is gated only by a 256KB transfer instead
    of the full first-chunk + weight-head serialization.
  - Endgame: the last two g layers are issued interleaved per tile, so the
    16 sigmoid+mul epilogues overlap the matmul stream instead of chaining
    after it. tanh/sigmoid/mul run in bf16 (2x DVE rate) and the output is
    DMA'd out as bf16 (half the bytes) in progressively smaller chunks, then
    widened to f32 on the host. bf16 rounding enters only in this final
    nonlinear stage, adding ~1e-3 relative error.
  - PE matmuls and HWDGE DMA descriptors only support a single sync wait and
    Tile's wait pass is not transitively minimal. The kernel therefore (a)
    warms the PE's vector clock with one tiny matmul per input-DMA lane so
    real matmuls never re-wait DMA lanes, and (b) keeps each input / output
    DMA stream on its own HWDGE lane with single-engine dependencies.
"""

import sys

import numpy as np

for _p in ("/opt/trn_rl_repo",):
    if _p not in sys.path:
        sys.path.append(_p)

B, T, C = 16, 8192, 64
K = 2
NUM_LAYERS = 13
N_CORES = 8
BPC = B // N_CORES          # batch elements per core
P = 2 * C                   # partitions used: (b, c) pairs
NTAP_TILE = 512             # tokens per matmul tile
NT = T // NTAP_TILE         # matmul tiles per layer
MARGIN = 256                # causal zero margin (covers dilations < 512)
NW = 2 * NUM_LAYERS * K     # packed conv weight count
W_HEAD = 8                  # weight tiles in the head DMA (first 4 layers run)
W_F0 = 2                    # f-l0's two weight tiles, DMA'd before x chunk 0

# x input chunks: small first chunk so layer-0 compute starts ASAP
XEDGE = [0, 512, 1536, 2560, 3584, 4608, 5632, 6656, 8192]
# output chunks (bf16), big early / small late so the final transfer is tiny
QEDGE = [0, 2048, 4096, 6144, 7168, 7680, 8192]
NQ = len(QEDGE) - 1

# layer execution schedule (branch, layer), chosen so BOTH branches' layer 0
# run during the x-chunk-paced startup (g-l0 parks its output in scratch,
# which is otherwise idle until f-l12), and weights are packed in this order
# so the head DMA covers exactly the first layers. The last two entries
# (g-l11, g-l12) are issued interleaved per tile by the tail loop.
SCHED = ([("f", 0), ("g", 0)] + [("f", l) for l in range(1, 12)] +
         [("g", 1), ("f", 12)] + [("g", l) for l in range(2, 13)])

_PROGRAM_CACHE = {}


def fp32r_bits(a):
    """Round f32 to fp32r (11-bit mantissa, RNE), low 12 bits zeroed."""
    u = np.ascontiguousarray(a, dtype=np.float32).view(np.uint32)
    keep = u >> np.uint32(12)
    low = u & np.uint32(0xFFF)
    rup = (low > 0x800) | ((low == 0x800) & ((keep & np.uint32(1)) == 1))
    return ((keep + rup.astype(np.uint32)) << np.uint32(12)).view(np.float32)


def _build_program():
    import concourse.bacc as bacc
    import concourse.tile as tile
    from concourse import mybir

    f32 = mybir.dt.float32
    f32r = mybir.dt.float32r
    bf16 = mybir.dt.bfloat16
    fp16 = mybir.dt.float16
    AF = mybir.ActivationFunctionType

    # Bacc (not bare Bass): its compile() legalizes multi-wait instructions
    # into event-semaphore chains (TRN2 allows 1 wait per instruction).
    nc = bacc.Bacc("TRN2", target_bir_lowering=False, debug=False)

    consts = nc.dram_tensor("consts", [P, 2 * P + 1], f32,
                            kind="ExternalInput").ap()
    xin = nc.dram_tensor("xr", [P, T], fp16, kind="ExternalInput").ap()
    wconv = nc.dram_tensor("wr", [P, NW * P], fp16, kind="ExternalInput").ap()
    outs = [nc.dram_tensor(f"out_q{q}", [P, QEDGE[q + 1] - QEDGE[q]], fp16,
                           kind="ExternalOutput").ap() for q in range(NQ)]

    with tile.TileContext(nc) as tc:
        with (
            tc.tile_pool(name="persist", bufs=1) as persist,
            tc.tile_pool(name="epool", bufs=4) as epool,
            tc.tile_pool(name="mpsum", bufs=8, space="PSUM") as mpsum,
        ):
            # ---- persistent buffers ---------------------------------------
            scratch = persist.tile([P, MARGIN + T], fp16, name="scratch",
                                   tag="scratch")
            call = persist.tile([P, 2 * P + 1], f32, name="call", tag="call")
            x0 = persist.tile([P, MARGIN + T], fp16, name="x0", tag="x0")
            bufA = persist.tile([P, MARGIN + T], fp16, name="bufA", tag="bufA")
            bufB = persist.tile([P, MARGIN + T], fp16, name="bufB", tag="bufB")
            wall = persist.tile([P, NW * P], fp16, name="wall", tag="wall")
            for buf in (x0, bufA, bufB, scratch):
                nc.vector.memset(buf[:, 0:MARGIN], 0.0)
            # dedicated bf16 buffers for the final nonlinear stage (the BIR
            # verifier rejects non-f32r-typed writes into any location an
            # f32r matmul reads, so these cannot alias x0/scratch)
            tanh16 = persist.tile([P, T], fp16, name="tanh16", tag="tanh16")
            out16 = persist.tile([P, T], fp16, name="out16", tag="out16")

            # ---- input DMAs, ordered by first consumption -----------------
            # consts -> f-l0 weights -> small x chunk 0 -> rest of weight
            # head -> remaining x chunks -> weight tail, so layer-0 compute
            # starts as early as the DMA wake-up latency allows.
            nc.sync.dma_start(
                out=x0[:, MARGIN + XEDGE[0]:MARGIN + XEDGE[1]],
                in_=xin[:, XEDGE[0]:XEDGE[1]])
            nc.sync.dma_start(out=wall[:, 0:W_F0 * P],
                              in_=wconv[:, 0:W_F0 * P])
            nc.sync.dma_start(out=call, in_=consts)
            nc.sync.dma_start(out=wall[:, W_F0 * P:W_HEAD * P],
                              in_=wconv[:, W_F0 * P:W_HEAD * P])
            for xc in range(1, len(XEDGE) - 1):
                nc.sync.dma_start(
                    out=x0[:, MARGIN + XEDGE[xc]:MARGIN + XEDGE[xc + 1]],
                    in_=xin[:, XEDGE[xc]:XEDGE[xc + 1]])
            nc.sync.dma_start(out=wall[:, W_HEAD * P:],
                              in_=wconv[:, W_HEAD * P:])

            # ---- PE lane warm-ups + h @ V biases --------------------------
            # single-shot matmuls into disjoint columns of one PSUM tile:
            # three consume the x / w-head0 / w-head DMA lanes (results
            # unused), then the two bias projections (their lane comes via
            # their own operands). lives in the conv-psum ring: only needed
            # in the first few us, so it must not pin a PSUM bank for the
            # whole kernel
            bias_ps = mpsum.tile([P, 8], f32, name="bias_ps", tag="mp")
            x0w = x0[:, MARGIN:MARGIN + 1]
            ww0 = wall[:, 0:1]
            ww1 = wall[:, W_F0 * P:W_F0 * P + 1]
            nc.tensor.matmul(bias_ps[0:1, 4:5], lhsT=x0w, rhs=x0w,
                             start=True, stop=True)
            nc.tensor.matmul(bias_ps[0:1, 5:6], lhsT=ww0, rhs=ww0,
                             start=True, stop=True)
            nc.tensor.matmul(bias_ps[0:1, 6:7], lhsT=ww1, rhs=ww1,
                             start=True, stop=True)
            h_t = call[:, 2 * P:2 * P + 1]
            bias = []
            for i in range(2):
                nc.tensor.matmul(bias_ps[:, i:i + 1],
                                 lhsT=call[:, i * P:(i + 1) * P],
                                 rhs=h_t, start=True, stop=True)
                bias_sb = persist.tile([P, 1], f32, name=f"bias{i}",
                                       tag=f"bias{i}")
                nc.vector.tensor_copy(bias_sb, bias_ps[:, i:i + 1])
                bias.append(bias_sb)

            # ---- conv stacks ----------------------------------------------
            drain_rr = [0]

            def conv_tile(src, layer, br, j, dst=None, fuse=None):
                d = 2 ** layer
                base = 2 * SCHED.index((br, layer))
                w0r = wall[:, base * P:(base + 1) * P]
                w1r = wall[:, (base + 1) * P:(base + 2) * P]
                t0 = j * NTAP_TILE
                # d < 512 boundary reads dip into the zero margin;
                # d >= 512 boundaries are tile-aligned -> tap 0 skipped.
                has_tap0 = t0 + NTAP_TILE > d
                ps = mpsum.tile([P, NTAP_TILE], f32,
                                name=f"ps_{br}{layer}_{j}", tag="mp")
                nc.tensor.matmul(
                    ps, lhsT=w1r,
                    rhs=src[:, MARGIN + t0:MARGIN + t0 + NTAP_TILE],
                    start=True, stop=not has_tap0)
                if has_tap0:
                    o0 = MARGIN + t0 - d
                    nc.tensor.matmul(
                        ps, lhsT=w0r, rhs=src[:, o0:o0 + NTAP_TILE],
                        start=False, stop=True)
                if fuse is None:
                    dslice = dst[:, MARGIN + t0:MARGIN + t0 + NTAP_TILE]
                    # alternate drains DVE/ACT so neither engine gates PE
                    # (GpSimd cannot read PSUM, so no 3-way rotation)
                    if drain_rr[0] % 2 == 1:
                        nc.scalar.copy(dslice, ps)
                    else:
                        nc.vector.tensor_copy(dslice, ps)
                    drain_rr[0] += 1
                else:
                    fuse(j, ps)

            def conv_layer(src, layer, br, dst=None, fuse=None):
                for j in range(NT):
                    conv_tile(src, layer, br, j, dst=dst, fuse=fuse)

            # f-l12's drain IS the tanh (ScalarE, fused bias), in bf16,
            # parked in scratch's head bytes for the g-l12 epilogue.
            def tanh_drain(j, ps_f):
                t0 = j * NTAP_TILE
                nc.scalar.activation(tanh16[:, t0:t0 + NTAP_TILE], ps_f,
                                     AF.Tanh, bias=bias[0])

            # g-l12 fused with the gating epilogue, all in bf16
            def epilogue(j, ps_g):
                t0 = j * NTAP_TILE
                sig = epool.tile([P, NTAP_TILE], fp16, name=f"sig{j}",
                                 tag="sig")
                nc.scalar.activation(sig, ps_g, AF.Sigmoid, bias=bias[1])
                nc.vector.tensor_mul(out16[:, t0:t0 + NTAP_TILE],
                                     tanh16[:, t0:t0 + NTAP_TILE], sig)

            # buffer rotation per SCHED:
            #   f: x0 -> A -> B -> A ... (f-l12 reads B, tanh -> scratch)
            #   g: x0 -> scratch -> A -> B ... (g-l12 reads A, epilogue)
            cur = {"f": x0, "g": x0}
            for br, layer in SCHED[:-2]:
                if (br, layer) == ("f", 12):
                    conv_layer(cur["f"], layer, br, fuse=tanh_drain)
                else:
                    if br == "f":
                        dst = bufA if layer % 2 == 0 else bufB
                    else:
                        dst = scratch if layer == 0 else \
                            (bufA if layer % 2 == 1 else bufB)
                    conv_layer(cur[br], layer, br, dst=dst)
                    cur[br] = dst

            # ---- tail: g-l11 and g-l12 interleaved per tile ---------------
            # g-l12 tile j only needs g-l11 tiles <= j (tap 0 reads 8 tiles
            # back), so a 1-tile lag spreads the 16 sigmoid+mul epilogues
            # across the final matmul stream instead of chaining after it.
            # Output chunks are DMA'd as soon as their last tile's mul is
            # issued (channel-major bf16; host widens and restores [b,t,c]).
            src11 = cur["g"]
            nxt_q = [0]

            def flush_outputs(jj):
                while (nxt_q[0] < NQ
                       and QEDGE[nxt_q[0] + 1] <= (jj + 1) * NTAP_TILE):
                    q = nxt_q[0]
                    nc.sync.dma_start(
                        out=outs[q], in_=out16[:, QEDGE[q]:QEDGE[q + 1]])
                    nxt_q[0] += 1

            for j in range(NT):
                conv_tile(src11, 11, "g", j, dst=bufA)
                if j >= 1:
                    conv_tile(bufA, 12, "g", j - 1, fuse=epilogue)
                    flush_outputs(j - 1)
            conv_tile(bufA, 12, "g", NT - 1, fuse=epilogue)
            flush_outputs(NT - 1)

    nc.compile()
    return nc


def get_program():
    if "nc" not in _PROGRAM_CACHE:
        _PROGRAM_CACHE["nc"] = _build_program()
    return _PROGRAM_CACHE["nc"]


def make_in_maps(x, h, Wf, Wg, Vf, Vg):
    x = np.asarray(x, dtype=np.float32)
    h = np.asarray(h, dtype=np.float32)
    eye2 = np.eye(2, dtype=np.float32)
    # SCHED-ordered [branch, layer, tap] -> kron(I2, W[tap]) as lhsT
    # [K=(b,cin), M=(b,cout)]
    Wn = {"f": np.asarray(Wf, dtype=np.float32),
          "g": np.asarray(Wg, dtype=np.float32)}
    wpack = np.zeros((NW, P, P), dtype=np.float32)
    for pos, (br, layer) in enumerate(SCHED):
        for tap in range(K):
            wpack[2 * pos + tap] = np.kron(eye2, Wn[br][layer, tap])
    # wall[p, i*P + m] = wpack[i, p, m], rounded to fp16 (10-bit mantissa;
    # conv weights are ~0.09 magnitude, well inside fp16 range)
    wcols = wpack.transpose(1, 0, 2).reshape(P, NW * P).astype(np.float16)
    vcat = np.concatenate(
        [np.kron(eye2, np.asarray(V, dtype=np.float32)) for V in (Vf, Vg)],
        axis=1)  # [128, 256]

    in_maps = []
    for core in range(N_CORES):
        sl = slice(core * BPC, (core + 1) * BPC)
        xcm = x[sl].transpose(0, 2, 1).reshape(P, T) \
            .astype(np.float16)  # [(b,c), t]
        consts = np.ascontiguousarray(
            np.concatenate([vcat, h[sl].reshape(P, 1)], axis=1))
        in_maps.append({"consts": consts, "xr": xcm, "wr": wcols})
    return in_maps


def _to_f32(a):
    """16-bit float (fp16, or bf16 in any container dtype) -> f32."""
    a = np.asarray(a)
    if a.dtype in (np.float32, np.float16):
        return a.astype(np.float32)
    u = a.view(np.uint16).astype(np.uint32) << np.uint32(16)
    return u.view(np.float32)


def assemble_output(results):
    full = np.empty((B, T, C), dtype=np.float32)
    for core, r in enumerate(results):
        cm = np.concatenate(
            [_to_f32(r[f"out_q{q}"]) for q in range(NQ)], axis=1)
        full[core * BPC:(core + 1) * BPC] = \
            cm.reshape(BPC, C, T).transpose(0, 2, 1)
    return full


def kernel(x, h, Wf, Wg, Vf, Vg):
    from concourse import bass_utils

    nc = get_program()
    in_maps = make_in_maps(x, h, Wf, Wg, Vf, Vg)
    res = bass_utils.run_bass_kernel_spmd(nc, in_maps,
                                          core_ids=list(range(N_CORES)))
    return assemble_output(res.results)


# revision 10
# speedup vs baseline: 1.0072x; 1.0072x over previous
"""Trainium2 Bass kernel for a 13-layer causal dilated conv stack with gating.

Model (per reference):
    Wx_f = 13 causal dilated convs (K=2, dilation 2^i) over x with Wf
    Wx_g = same with Wg
    out  = tanh(Wx_f + h@Vf) * sigmoid(Wx_g + h@Vg)

Shapes: x (16, 8192, 64) f32, h (16, 64), Wf/Wg (13, 2, 64, 64), Vf/Vg (64, 64).

Strategy:
  - Data-parallel over batch: 2 batch elements per core on 8 cores, no
    collectives.
  - On-chip layout [128 partitions = (b*64 + c), T free]: both local batch
    elements' channels stacked on the partition axis. The host pre-transposes
    x into this channel-major layout (and transposes the output back), so the
    device does no layout changes at all.
  - Each conv layer is, per 512-token tile, two accumulating PE matmuls (one
    per tap) with block-diagonal kron(I2, W[tap]) stationary weights -> full
    128-wide PE utilization.
  - Causality: activation buffers carry a 256-column zero margin covering
    dilations < 512; for d >= 512 the boundary is tile-aligned and the tap-0
    matmul is simply skipped on the first d/512 tiles.
  - Full fp16 datapath (x, weights, inter-layer activations): full-rate PE
    like fp32r, but halves LDWEIGHTS SBUF traffic (the per-matmul weight
    reload is fully hidden at ~216ns matmul spacing vs ~227ns for f32r),
    halves input DMA, and keeps 10-bit mantissas so the 26-layer stack
    stays at ~4e-3 relative error (bf16's 8-bit mantissa fails the gate).
    PSUM accumulation is f32 throughout; mixing 16/32-bit matmul operands
    is not supported by the backend, so weights and activations must both
    be fp16.
  - h@V bias is computed on-device with kron(I2, V) and fused into the
    tanh/sigmoid activations via the ScalarE bias port.
  - PSUM->SBUF drain copies alternate DVE/DVE/ACT so neither engine
    bottlenecks the PE (scratch stays DVE-only for the output DMAs).
  - Startup: DMA issue order is consts -> f0 weights -> a small (512 col)
    first x chunk -> rest of the weight head -> remaining x chunks -> weight
    tail, so the first conv matmul is gated only by a 256KB transfer instead
    of the full first-chunk + weight-head serialization.
  - Endgame: the last two g layers are issued interleaved per tile, so the
    16 sigmoid+mul epilogues overlap the matmul stream instead of chaining
    after it. tanh/sigmoid/mul run in bf16 (2x DVE rate) and the output is
    DMA'd out as bf16 (half the bytes) in progressively smaller chunks, then
    widened to f32 on the host. bf16 rounding enters only in this final
    nonlinear stage, adding ~1e-3 relative error.
  - PE matmuls and HWDGE DMA descriptors only support a single sync wait and
    Tile's wait pass is not transitively minimal. The kernel therefore (a)
    warms the PE's vector clock with one tiny matmul per input-DMA lane so
    real matmuls never re-wait DMA lanes, and (b) keeps each input / output
    DMA stream on its own HWDGE lane with single-engine dependencies.
"""

import sys

import numpy as np

for _p in ("/opt/trn_rl_repo",):
    if _p not in sys.path:
        sys.path.append(_p)

B, T, C = 16, 8192, 64
K = 2
NUM_LAYERS = 13
N_CORES = 8
BPC = B // N_CORES          # batch elements per core
P = 2 * C                   # partitions used: (b, c) pairs
NTAP_TILE = 512             # tokens per matmul tile
NT = T // NTAP_TILE         # matmul tiles per layer
MARGIN = 256                # causal zero margin (covers dilations < 512)
NW = 2 * NUM_LAYERS * K     # packed conv weight count
W_HEAD = 8                  # weight tiles in the head DMA (first 4 layers run)
W_F0 = 2                    # f-l0's two weight tiles, DMA'd before x chunk 0

# x input chunks: small first chunk so layer-0 compute starts ASAP
XEDGE = [0, 512, 1536, 2560, 3584, 4608, 5632, 6656, 8192]
# output chunks (bf16), big early / small late so the final transfer is tiny
QEDGE = [0, 2048, 4096, 6144, 7168, 7680, 8192]
NQ = len(QEDGE) - 1

# layer execution schedule (branch, layer), chosen so BOTH branches' layer 0
# run during the x-chunk-paced startup (g-l0 parks its output in scratch,
# which is otherwise idle until f-l12), and weights are packed in this order
# so the head DMA covers exactly the first layers. The last two entries
# (g-l11, g-l12) are issued interleaved per tile by the tail loop.
SCHED = ([("f", 0), ("g", 0)] + [("f", l) for l in range(1, 12)] +
         [("g", 1), ("f", 12)] + [("g", l) for l in range(2, 13)])

_PROGRAM_CACHE = {}


def fp32r_bits(a):
    """Round f32 to fp32r (11-bit mantissa, RNE), low 12 bits zeroed."""
    u = np.ascontiguousarray(a, dtype=np.float32).view(np.uint32)
    keep = u >> np.uint32(12)
    low = u & np.uint32(0xFFF)
    rup = (low > 0x800) | ((low == 0x800) & ((keep & np.uint32(1)) == 1))
    return ((keep + rup.astype(np.uint32)) << np.uint32(12)).view(np.float32)


def _build_program():
    import concourse.bacc as bacc
    import concourse.tile as tile
    from concourse import mybir

    f32 = mybir.dt.float32
    f32r = mybir.dt.float32r
    bf16 = mybir.dt.bfloat16
    fp16 = mybir.dt.float16
    AF = mybir.ActivationFunctionType

    # Bacc (not bare Bass): its compile() legalizes multi-wait instructions
    # into event-semaphore chains (TRN2 allows 1 wait per instruction).
    nc = bacc.Bacc("TRN2", target_bir_lowering=False, debug=False)

    consts = nc.dram_tensor("consts", [P, 2 * P + 1], f32,
                            kind="ExternalInput").ap()
    xin = nc.dram_tensor("xr", [P, T], fp16, kind="ExternalInput").ap()
    wconv = nc.dram_tensor("wr", [P, NW * P], fp16, kind="ExternalInput").ap()
    outs = [nc.dram_tensor(f"out_q{q}", [P, QEDGE[q + 1] - QEDGE[q]], fp16,
                           kind="ExternalOutput").ap() for q in range(NQ)]

    with tile.TileContext(nc) as tc:
        with (
            tc.tile_pool(name="persist", bufs=1) as persist,
            tc.tile_pool(name="epool", bufs=4) as epool,
            tc.tile_pool(name="mpsum", bufs=8, space="PSUM") as mpsum,
        ):
            # ---- persistent buffers ---------------------------------------
            scratch = persist.tile([P, MARGIN + T], fp16, name="scratch",
                                   tag="scratch")
            call = persist.tile([P, 2 * P + 1], f32, name="call", tag="call")
            x0 = persist.tile([P, MARGIN + T], fp16, name="x0", tag="x0")
            bufA = persist.tile([P, MARGIN + T], fp16, name="bufA", tag="bufA")
            bufB = persist.tile([P, MARGIN + T], fp16, name="bufB", tag="bufB")
            wall = persist.tile([P, NW * P], fp16, name="wall", tag="wall")
            for buf in (x0, bufA, bufB, scratch):
                nc.vector.memset(buf[:, 0:MARGIN], 0.0)
            # dedicated bf16 buffers for the final nonlinear stage (the BIR
            # verifier rejects non-f32r-typed writes into any location an
            # f32r matmul reads, so these cannot alias x0/scratch)
            tanh16 = persist.tile([P, T], fp16, name="tanh16", tag="tanh16")
            out16 = persist.tile([P, T], fp16, name="out16", tag="out16")

            # ---- input DMAs, ordered by first consumption -----------------
            # consts -> f-l0 weights -> small x chunk 0 -> rest of weight
            # head -> remaining x chunks -> weight tail, so layer-0 compute
            # starts as early as the DMA wake-up latency allows.
            nc.sync.dma_start(
                out=x0[:, MARGIN + XEDGE[0]:MARGIN + XEDGE[1]],
                in_=xin[:, XEDGE[0]:XEDGE[1]])
            nc.sync.dma_start(out=wall[:, 0:W_F0 * P],
                              in_=wconv[:, 0:W_F0 * P])
            nc.sync.dma_start(out=call, in_=consts)
            nc.sync.dma_start(out=wall[:, W_F0 * P:W_HEAD * P],
                              in_=wconv[:, W_F0 * P:W_HEAD * P])
            for xc in range(1, len(XEDGE) - 1):
                nc.sync.dma_start(
                    out=x0[:, MARGIN + XEDGE[xc]:MARGIN + XEDGE[xc + 1]],
                    in_=xin[:, XEDGE[xc]:XEDGE[xc + 1]])
            nc.sync.dma_start(out=wall[:, W_HEAD * P:],
                              in_=wconv[:, W_HEAD * P:])

            # ---- PE lane warm-ups + h @ V biases --------------------------
            # single-shot matmuls into disjoint columns of one PSUM tile:
            # three consume the x / w-head0 / w-head DMA lanes (results
            # unused), then the two bias projections (their lane comes via
            # their own operands). lives in the conv-psum ring: only needed
            # in the first few us, so it must not pin a PSUM bank for the
            # whole kernel
            bias_ps = mpsum.tile([P, 8], f32, name="bias_ps", tag="mp")
            x0w = x0[:, MARGIN:MARGIN + 1]
            ww0 = wall[:, 0:1]
            ww1 = wall[:, W_F0 * P:W_F0 * P + 1]
            nc.tensor.matmul(bias_ps[0:1, 4:5], lhsT=x0w, rhs=x0w,
                             start=True, stop=True)
            nc.tensor.matmul(bias_ps[0:1, 5:6], lhsT=ww0, rhs=ww0,
                             start=True, stop=True)
            nc.tensor.matmul(bias_ps[0:1, 6:7], lhsT=ww1, rhs=ww1,
                             start=True, stop=True)
            h_t = call[:, 2 * P:2 * P + 1]
            bias = []
            for i in range(2):
                nc.tensor.matmul(bias_ps[:, i:i + 1],
                                 lhsT=call[:, i * P:(i + 1) * P],
                                 rhs=h_t, start=True, stop=True)
                bias_sb = persist.tile([P, 1], f32, name=f"bias{i}",
                                       tag=f"bias{i}")
                nc.vector.tensor_copy(bias_sb, bias_ps[:, i:i + 1])
                bias.append(bias_sb)

            # ---- conv stacks ----------------------------------------------
            drain_rr = [0]

            def conv_tile(src, layer, br, j, dst=None, fuse=None):
                d = 2 ** layer
                base = 2 * SCHED.index((br, layer))
                w0r = wall[:, base * P:(base + 1) * P]
                w1r = wall[:, (base + 1) * P:(base + 2) * P]
                t0 = j * NTAP_TILE
                # d < 512 boundary reads dip into the zero margin;
                # d >= 512 boundaries are tile-aligned -> tap 0 skipped.
                has_tap0 = t0 + NTAP_TILE > d
                ps = mpsum.tile([P, NTAP_TILE], f32,
                                name=f"ps_{br}{layer}_{j}", tag="mp")
                nc.tensor.matmul(
                    ps, lhsT=w1r,
                    rhs=src[:, MARGIN + t0:MARGIN + t0 + NTAP_TILE],
                    start=True, stop=not has_tap0)
                if has_tap0:
                    o0 = MARGIN + t0 - d
                    nc.tensor.matmul(
                        ps, lhsT=w0r, rhs=src[:, o0:o0 + NTAP_TILE],
                        start=False, stop=True)
                if fuse is None:
                    dslice = dst[:, MARGIN + t0:MARGIN + t0 + NTAP_TILE]
                    # alternate drains DVE/ACT so neither engine gates PE
                    # (GpSimd cannot read PSUM, so no 3-way rotation)
                    if drain_rr[0] % 2 == 1:
                        nc.scalar.copy(dslice, ps)
                    else:
                        nc.vector.tensor_copy(dslice, ps)
                    drain_rr[0] += 1
                else:
                    fuse(j, ps)

            def conv_layer(src, layer, br, dst=None, fuse=None):
                for j in range(NT):
                    conv_tile(src, layer, br, j, dst=dst, fuse=fuse)

            # f-l12's drain IS the tanh (ScalarE, fused bias), in bf16,
            # parked in scratch's head bytes for the g-l12 epilogue.
            def tanh_drain(j, ps_f):
                t0 = j * NTAP_TILE
                nc.scalar.activation(tanh16[:, t0:t0 + NTAP_TILE], ps_f,
                                     AF.Tanh, bias=bias[0])

            # g-l12 fused with the gating epilogue, all in bf16
            def epilogue(j, ps_g):
                t0 = j * NTAP_TILE
                sig = epool.tile([P, NTAP_TILE], fp16, name=f"sig{j}",
                                 tag="sig")
                nc.scalar.activation(sig, ps_g, AF.Sigmoid, bias=bias[1])
                nc.vector.tensor_mul(out16[:, t0:t0 + NTAP_TILE],
                                     tanh16[:, t0:t0 + NTAP_TILE], sig)

            # buffer rotation per SCHED:
            #   f: x0 -> A -> B -> A ... (f-l12 reads B, tanh -> scratch)
            #   g: x0 -> scratch -> A -> B ... (g-l12 reads A, epilogue)
            cur = {"f": x0, "g": x0}
            for br, layer in SCHED[:-2]:
                if (br, layer) == ("f", 12):
                    conv_layer(cur["f"], layer, br, fuse=tanh_drain)
                else:
                    if br == "f":
                        dst = bufA if layer % 2 == 0 else bufB
                    else:
                        dst = scratch if layer == 0 else \
                            (bufA if layer % 2 == 1 else bufB)
                    conv_layer(cur[br], layer, br, dst=dst)
                    cur[br] = dst

            # ---- tail: g-l11 and g-l12 interleaved per tile ---------------
            # g-l12 tile j only needs g-l11 tiles <= j (tap 0 reads 8 tiles
            # back), so a 2-tile lag spreads the 16 sigmoid+mul epilogues
            # across the final matmul stream instead of chaining after it.
            # Output chunks are DMA'd as soon as their last tile's mul is
            # issued (channel-major bf16; host widens and restores [b,t,c]).
            src11 = cur["g"]
            nxt_q = [0]

            def flush_outputs(jj):
                while (nxt_q[0] < NQ
                       and QEDGE[nxt_q[0] + 1] <= (jj + 1) * NTAP_TILE):
                    q = nxt_q[0]
                    nc.sync.dma_start(
                        out=outs[q], in_=out16[:, QEDGE[q]:QEDGE[q + 1]])
                    nxt_q[0] += 1

            for j in range(NT):
                conv_tile(src11, 11, "g", j, dst=bufA)
                if j >= 2:
                    conv_tile(bufA, 12, "g", j - 2, fuse=epilogue)
                    flush_outputs(j - 2)
            for j in (NT - 2, NT - 1):
                conv_tile(bufA, 12, "g", j, fuse=epilogue)
                flush_outputs(j)

    nc.compile()
    return nc


def get_program():
    if "nc" not in _PROGRAM_CACHE:
        _PROGRAM_CACHE["nc"] = _build_program()
    return _PROGRAM_CACHE["nc"]


def make_in_maps(x, h, Wf, Wg, Vf, Vg):
    x = np.asarray(x, dtype=np.float32)
    h = np.asarray(h, dtype=np.float32)
    eye2 = np.eye(2, dtype=np.float32)
    # SCHED-ordered [branch, layer, tap] -> kron(I2, W[tap]) as lhsT
    # [K=(b,cin), M=(b,cout)]
    Wn = {"f": np.asarray(Wf, dtype=np.float32),
          "g": np.asarray(Wg, dtype=np.float32)}
    wpack = np.zeros((NW, P, P), dtype=np.float32)
    for pos, (br, layer) in enumerate(SCHED):
        for tap in range(K):
            wpack[2 * pos + tap] = np.kron(eye2, Wn[br][layer, tap])
    # wall[p, i*P + m] = wpack[i, p, m], rounded to fp16 (10-bit mantissa;
    # conv weights are ~0.09 magnitude, well inside fp16 range)
    wcols = wpack.transpose(1, 0, 2).reshape(P, NW * P).astype(np.float16)
    vcat = np.concatenate(
        [np.kron(eye2, np.asarray(V, dtype=np.float32)) for V in (Vf, Vg)],
        axis=1)  # [128, 256]

    in_maps = []
    for core in range(N_CORES):
        sl = slice(core * BPC, (core + 1) * BPC)
        xcm = x[sl].transpose(0, 2, 1).reshape(P, T) \
            .astype(np.float16)  # [(b,c), t]
        consts = np.ascontiguousarray(
            np.concatenate([vcat, h[sl].reshape(P, 1)], axis=1))
        in_maps.append({"consts": consts, "xr": xcm, "wr": wcols})
    return in_maps


def _to_f32(a):
    """16-bit float (fp16, or bf16 in any container dtype) -> f32."""
    a = np.asarray(a)
    if a.dtype in (np.float32, np.float16):
        return a.astype(np.float32)
    u = a.view(np.uint16).astype(np.uint32) << np.uint32(16)
    return u.view(np.float32)


def assemble_output(results):
    full = np.empty((B, T, C), dtype=np.float32)
    for core, r in enumerate(results):
        cm = np.concatenate(
            [_to_f32(r[f"out_q{q}"]) for q in range(NQ)], axis=1)
        full[core * BPC:(core + 1) * BPC] = \
            cm.reshape(BPC, C, T).transpose(0, 2, 1)
    return full


def kernel(x, h, Wf, Wg, Vf, Vg):
    from concourse import bass_utils

    nc = get_program()
    in_maps = make_in_maps(x, h, Wf, Wg, Vf, Vg)
    res = bass_utils.run_bass_kernel_spmd(nc, in_maps,
                                          core_ids=list(range(N_CORES)))
    return assemble_output(res.results)
